# revision 11
# baseline (speedup 1.0000x reference)
"""Trainium2 Bass kernel for the Nunchaku Qwen-Image transformer block.

8-way tensor-parallel over attention heads / FFN columns, with
Megatron-style sequence parallelism for the norm/modulate phases
(reduce-scatter + all-gather instead of all-reduce).  All matmuls run in
bf16 with fp32 PSUM accumulation; weights are pre-cast / pre-sharded /
pre-transposed on the host (host prep is outside the measured HW time).

Device-side token orders:
  * "permuted" order (the RS/AG shard order): core c owns rows
    [32 txt tokens 32c:32c+32 | 128 img tokens 128c:128(c+1)].
  * "tile" order == original [txt | img] concat order; used inside
    attention.  img tiles are contiguous in both orders; txt tiles map to
    4 blocks of 32 rows in permuted order.
"""

import sys
import numpy as np

sys.path.insert(0, "/opt/trn_rl_repo")

import ml_dtypes

BF16 = ml_dtypes.bfloat16

# ---------------------------------------------------------------- constants
B, S_TXT, S_IMG, D, H, HD, FF = 1, 256, 1024, 3072, 24, 128, 12288
S = S_TXT + S_IMG            # 1280
NC = 8
HL = H // NC                 # 3 heads / core
FFL = FF // NC               # 1536 ffn cols / core
CHL = D // NC                # 384 qkv dims per matrix / core
R = S // NC                  # 160 seq rows / core shard
EPS = 1e-6
NKT = D // 128               # 24 k-tiles over D
NF = FFL // 128              # 12 ff k-tiles
NT_TXT, NT = 2, 10           # 128-token tiles (2 txt + 8 img)
INV_SQRT_HD = 1.0 / float(HD) ** 0.5

HD_PERM = np.concatenate([np.arange(0, HD, 2), np.arange(1, HD, 2)])

SEQ_PERM = np.concatenate(
    [np.concatenate([np.arange(32 * c, 32 * c + 32),
                     S_TXT + np.arange(128 * c, 128 * c + 128)])
     for c in range(NC)])  # perm_pos -> concat idx


# ------------------------------------------------------------- device program
def _build_program(sim=False, single=False):
    from contextlib import ExitStack

    import concourse.bass as bass
    import concourse.tile as tile
    from concourse import bacc, mybir

    f32 = mybir.dt.float32
    bf16 = mybir.dt.bfloat16
    AF = mybir.ActivationFunctionType
    OP = mybir.AluOpType
    AX = mybir.AxisListType

    nc = bacc.Bacc("TRN2", target_bir_lowering=False, debug=False,
                   enable_asserts=False, num_devices=1 if single else NC)

    def din(name, shape, dt=f32):
        return nc.dram_tensor(name, list(shape), dt, kind="ExternalInput")

    h_shard = din("h_shard", [R, D])
    temb_in = din("temb_in", [D])
    ropec = din("ropec", [S, 64])
    ropes = din("ropes", [S, 64])
    mask_b = din("mask_b", [S])
    mod_wT = din("mod_wT", [D, 4608], bf16)
    mod_b = din("mod_b", [4608])
    qkv_wT = din("qkv_wT", [D, 3 * CHL], bf16)
    qkv_b = din("qkv_b", [3 * CHL])
    aqkv_wT = din("aqkv_wT", [D, 3 * CHL], bf16)
    aqkv_b = din("aqkv_b", [3 * CHL])
    nqw = din("nqw", [HD]); nkw = din("nkw", [HD])
    naqw = din("naqw", [HD]); nakw = din("nakw", [HD])
    toout_wT = din("toout_wT", [CHL, D], bf16)
    taout_wT = din("taout_wT", [CHL, D], bf16)
    toout_b = din("toout_b", [D]); taout_b = din("taout_b", [D])
    w1iT = din("w1iT", [D, FFL], bf16); w1tT = din("w1tT", [D, FFL], bf16)
    b1i = din("b1i", [FFL]); b1t = din("b1t", [FFL])
    w2iT = din("w2iT", [FFL, D], bf16); w2tT = din("w2tT", [FFL, D], bf16)
    b2i = din("b2i", [D]); b2t = din("b2t", [D])

    out_shard = nc.dram_tensor("out_shard", [R, D], f32, kind="ExternalOutput")

    RG = [list(range(NC))]

    def collective(kind, op, in_t, out_t):
        if single:
            # timing-model stand-in: local DMA touching both buffers
            n = min(in_t[:].size(), out_t[:].size())
            nc.gpsimd.dma_start(
                out=bass.AP(tensor=out_t[:].tensor, offset=out_t[:].offset,
                            ap=[[1, n]]),
                in_=bass.AP(tensor=in_t[:].tensor, offset=in_t[:].offset,
                            ap=[[1, n]]))
            return
        nc.gpsimd.collective_compute(kind, op, replica_groups=RG,
                                     ins=[in_t.opt()], outs=[out_t.opt()])

    def bc_ap(handle_or_ap, offset, ap):
        base = handle_or_ap[:]
        return bass.AP(tensor=base.tensor, offset=base.offset + offset, ap=ap)

    ctx = ExitStack()
    with tile.TileContext(nc) as tc, ctx:
        pool = tc.tile_pool
        dram = ctx.enter_context(pool(name="dram", bufs=1, space="DRAM"))
        consts = ctx.enter_context(pool(name="consts", bufs=1))
        small = ctx.enter_context(pool(name="small", bufs=2))
        wstream = ctx.enter_context(pool(name="wstream", bufs=6))

        # collective bounce buffers
        mod_ag_in = dram.tile([4608], f32)
        mod_ag_out = dram.tile([NC, 4608], f32, addr_space="Shared")
        x1_ag_in = dram.tile([R, D], bf16)
        x1_full = dram.tile([S, D], bf16, addr_space="Shared")
        attn_rs_in = dram.tile([S, D], bf16)
        attn_rs_out = dram.tile([R, D], bf16)
        x2_ag_in = dram.tile([R, D], bf16)
        x2_full = dram.tile([S, D], bf16, addr_space="Shared")
        mlp_rs_in = dram.tile([S, D], bf16)
        mlp_rs_out = dram.tile([R, D], bf16)

        def tile_rows(dram_t, t, csl=slice(0, D)):
            """row-AP of DRAM [S, D] (permuted order) for tile t."""
            if t >= NT_TXT:
                m = t - NT_TXT
                return dram_t[160 * m + 32: 160 * m + 160, csl]
            v = dram_t[:].rearrange("(c r) x -> c r x", c=NC)
            return v[4 * t: 4 * t + 4, 0:32, csl]

        def xT_cols(xT, xTt, k, t):
            """contiguous column-AP of L2 activations for tile t."""
            if t >= NT_TXT:
                m = t - NT_TXT
                return xT[:, k, 160 * m + 32: 160 * m + 160]
            return xTt[:, k, 128 * t: 128 * (t + 1)]

        def gather_txt(xT, xTt):
            # txt tokens live in 8 blocks of 32 permuted cols; gather them
            # into a contiguous [128, 256] per k-tile (matmul lhsT needs a
            # single free dim).
            for k in range(NKT):
                nc.vector.tensor_copy(
                    xTt[:, k].rearrange("p (c r) -> p c r", c=NC),
                    xT[:, k].rearrange("p (c r) -> p c r", c=NC)[:, :, 0:32])

        tl_eps = consts.tile([128, 1], f32)
        nc.vector.memset(tl_eps, EPS)
        tl_mb = consts.tile([128, NT], f32)
        nc.gpsimd.dma_start(out=tl_mb,
                            in_=bc_ap(mask_b, 0, [[1, 128], [128, NT]]))

        def layernorm(dst, src, parts):
            stats = small.tile([parts, 6, 6], f32, tag="lnstats")
            for g in range(6):
                nc.vector.bn_stats(stats[:, g], src[:, 512 * g:512 * (g + 1)])
            mv = small.tile([parts, 2], f32, tag="lnmv")
            nc.vector.bn_aggr(mv, stats)
            rstd = small.tile([parts, 1], f32, tag="lnrstd")
            nc.scalar.activation(rstd, mv[:, 1:2], AF.Sqrt, bias=tl_eps[:parts])
            nc.vector.reciprocal(rstd, rstd)
            nc.vector.tensor_scalar(dst, src, mv[:, 0:1], rstd,
                                    OP.subtract, OP.mult)

        def plane_bcast(bpool, stream, p, parts=128):
            """modulation plane (stream 0=img,1=txt; p 0..5) -> [P, D] bf16."""
            t = bpool.tile([parts, D], bf16, tag="plane")
            off = 2304 * stream + CHL * p
            nc.gpsimd.dma_start(
                out=t.rearrange("q (c j) -> q c j", c=NC),
                in_=bc_ap(mod_ag_out, off, [[0, parts], [4608, NC], [1, CHL]]))
            return t

        def vec_bcast(bpool, dram_vec, parts=128):
            n = dram_vec.shape[0]
            t = bpool.tile([parts, n], bf16, tag="vb")
            nc.gpsimd.dma_start(out=t,
                                in_=bc_ap(dram_vec, 0, [[0, parts], [1, n]]))
            return t

        def modulate(spool, dst_bf, x, scale_t, shift_t, parts):
            tmp = spool.tile([parts, D], f32, tag="big32", bufs=2, name="modtmp")
            nc.vector.tensor_tensor(tmp, x, scale_t[:parts], OP.mult)
            nc.vector.tensor_tensor(dst_bf, tmp, shift_t[:parts], OP.add)

        # =================== P0 + P1 ====================================
        with pool(name="p01", bufs=1) as p01, \
             pool(name="psum0", bufs=2, space="PSUM") as psum0:
            tl_temb = small.tile([128, NKT], f32, tag="temb")
            nc.gpsimd.dma_start(out=tl_temb,
                                in_=bc_ap(temb_in, 0, [[1, 128], [128, NKT]]))
            tl_sg = small.tile([128, NKT], f32, tag="sg")
            nc.scalar.activation(tl_sg, tl_temb, AF.Sigmoid)
            tl_silu = small.tile([128, NKT], bf16, tag="silu")
            nc.vector.tensor_tensor(tl_silu, tl_temb, tl_sg, OP.mult)

            tl_modb = p01.tile([1, 4608], f32)
            nc.gpsimd.dma_start(out=tl_modb, in_=mod_b[None, :])
            tl_mloc = p01.tile([1, 4608], f32)
            for n in range(9):
                pm = psum0.tile([1, 512], f32, tag="pm")
                for k in range(NKT):
                    wt = wstream.tile([128, 512], bf16, tag="w")
                    nc.sync.dma_start(
                        out=wt, in_=mod_wT[128 * k:128 * (k + 1),
                                           512 * n:512 * (n + 1)])
                    nc.tensor.matmul(pm, tl_silu[:, k:k + 1], wt,
                                     start=(k == 0), stop=(k == NKT - 1))
                nc.vector.tensor_tensor(tl_mloc[:, 512 * n:512 * (n + 1)], pm,
                                        tl_modb[:, 512 * n:512 * (n + 1)],
                                        OP.add)
            for off in (CHL, 4 * CHL, 2304 + CHL, 2304 + 4 * CHL):
                nc.scalar.add(tl_mloc[:, off:off + CHL],
                              tl_mloc[:, off:off + CHL], 1.0)
            nc.gpsimd.dma_start(out=mod_ag_in[None, :], in_=tl_mloc)
            collective("AllGather", OP.bypass, mod_ag_in, mod_ag_out)

            # P1: LN1 + modulate shard -> AG x1
            for (parts, rsl, stream) in ((32, slice(0, 32), 1),
                                         (128, slice(32, 160), 0)):
                hsh = p01.tile([parts, D], f32, tag="big32b", bufs=2, name="hsh1")
                nc.gpsimd.dma_start(out=hsh, in_=h_shard[rsl])
                lnx = p01.tile([parts, D], f32, tag="big32c", bufs=2, name="lnx1")
                layernorm(lnx, hsh, parts)
                sc = plane_bcast(p01, stream, 1, parts)
                sh = plane_bcast(p01, stream, 0, parts)
                x1b = p01.tile([parts, D], bf16, tag="bigbf", bufs=2, name="x1b")
                modulate(p01, x1b, lnx, sc, sh, parts)
                nc.gpsimd.dma_start(out=x1_ag_in[rsl], in_=x1b)
            collective("AllGather", OP.bypass, x1_ag_in, x1_full)

        # =================== attention scope ============================
        with pool(name="attnp", bufs=1) as attnp:
            tl_QT = attnp.tile([128, HL, S], bf16)
            tl_KT = attnp.tile([128, HL, S], bf16)
            tl_V = attnp.tile([128, NT, HL, HD + 1], bf16)
            tl_attn = attnp.tile([128, NT, CHL], bf16)
            tl_attnT = attnp.tile([128, HL, S], bf16)

            # ---------------- P2 + P3 ----------------
            with pool(name="p2", bufs=1) as p2, \
                 pool(name="psum2", bufs=8, space="PSUM") as psum2:
                tl_xT = p2.tile([128, NKT, S], bf16)
                for k in range(NKT):
                    nc.sync.dma_start(out=tl_xT[:, k],
                                      in_=x1_full[:, 128 * k:128 * (k + 1)],
                                      transpose=True)
                tl_xTt = p2.tile([128, NKT, 256], bf16)
                gather_txt(tl_xT, tl_xTt)

                tl_nw = p2.tile([128, 2, HL, HD], bf16)
                tl_naw = p2.tile([128, 2, HL, HD], bf16)
                for (dst, qsrc, ksrc) in ((tl_nw, nqw, nkw), (tl_naw, naqw, nakw)):
                    for hh in range(HL):
                        nc.gpsimd.dma_start(
                            out=dst[:, 0, hh],
                            in_=bc_ap(qsrc, 0, [[0, 128], [1, HD]]))
                        nc.gpsimd.dma_start(
                            out=dst[:, 1, hh],
                            in_=bc_ap(ksrc, 0, [[0, 128], [1, HD]]))
                tl_qkvb = p2.tile([128, 3 * CHL], bf16)
                nc.gpsimd.dma_start(
                    out=tl_qkvb, in_=bc_ap(qkv_b, 0, [[0, 128], [1, 3 * CHL]]))
                tl_aqkvb = p2.tile([128, 3 * CHL], bf16)
                nc.gpsimd.dma_start(
                    out=tl_aqkvb, in_=bc_ap(aqkv_b, 0, [[0, 128], [1, 3 * CHL]]))

                tl_qkvS = p2.tile([128, NT, 3 * CHL], bf16)  # qkv storage

                for stream in (0, 1):  # img, txt
                    wT = qkv_wT if stream == 0 else aqkv_wT
                    bias_t = tl_qkvb if stream == 0 else tl_aqkvb
                    tiles = list(range(NT_TXT, NT)) if stream == 0 \
                        else list(range(NT_TXT))
                    for nn in range(3):
                        nsl = slice(384 * nn, 384 * (nn + 1))
                        pqs = [psum2.tile([128, 384], f32, tag="pq",
                                          name=f"pq_{stream}_{nn}_{i}")
                               for i in range(len(tiles))]
                        for k in range(NKT):
                            wt = wstream.tile([128, 384], bf16, tag="w")
                            nc.sync.dma_start(out=wt,
                                              in_=wT[128 * k:128 * (k + 1), nsl])
                            for i, t in enumerate(tiles):
                                nc.tensor.matmul(pqs[i], xT_cols(tl_xT, tl_xTt, k, t),
                                                 wt, start=(k == 0),
                                                 stop=(k == NKT - 1))
                        for i, t in enumerate(tiles):
                            nc.vector.tensor_tensor(tl_qkvS[:, t, nsl], pqs[i],
                                                    bias_t[:, nsl], OP.add)

                # ---- P3 per tile: rms + rope + transposes
                for t in range(NT):
                    txt = t < NT_TXT
                    qk = tl_qkvS[:, t, :2 * CHL]
                    sq = p2.tile([128, 2 * CHL], f32, tag="sq", bufs=2, name="sq")
                    nc.scalar.activation(sq, qk, AF.Square)
                    sums = small.tile([128, 2 * HL], f32, tag="sums")
                    for j in range(2 * HL):
                        nc.vector.reduce_sum(sums[:, j:j + 1],
                                             sq[:, HD * j:HD * (j + 1)],
                                             axis=AX.X)
                    rstd = small.tile([128, 2 * HL], f32, tag="rstd")
                    nc.scalar.activation(rstd, sums, AF.Sqrt, scale=1.0 / HD,
                                         bias=tl_eps)
                    nc.vector.reciprocal(rstd, rstd)
                    nw = tl_naw if txt else tl_nw
                    qkn = p2.tile([128, 2, HL, HD], f32, tag="qkn", bufs=2, name="qkn")
                    for j in range(2 * HL):
                        nc.vector.scalar_tensor_tensor(
                            qkn[:, j // HL, j % HL], qk[:, HD * j:HD * (j + 1)],
                            rstd[:, j:j + 1], nw[:, j // HL, j % HL],
                            OP.mult, OP.mult)
                    cs = small.tile([128, 2, 2 * HL, 64], f32, tag="cs")
                    for (ci, src) in ((0, ropec), (1, ropes)):
                        nc.gpsimd.dma_start(
                            out=cs[:, ci],
                            in_=bc_ap(src, 128 * t * 64,
                                      [[64, 128], [0, 2 * HL], [1, 64]]))
                    qkn4 = qkn.rearrange("p a h (u x) -> p (a h) u x", u=2)
                    qkr = p2.tile([128, 2, HL, HD], bf16, tag="qkr", bufs=2)
                    qkr4 = qkr.rearrange("p a h (u x) -> p (a h) u x", u=2)
                    ta = p2.tile([128, 2 * HL, 64], f32, tag="ropet", bufs=2)
                    tb = p2.tile([128, 2 * HL, 64], f32, tag="ropet2", bufs=2)
                    nc.vector.tensor_tensor(ta, qkn4[:, :, 0], cs[:, 0], OP.mult)
                    nc.vector.tensor_tensor(tb, qkn4[:, :, 1], cs[:, 1], OP.mult)
                    nc.vector.tensor_tensor(qkr4[:, :, 0], ta, tb, OP.subtract)
                    ta2 = p2.tile([128, 2 * HL, 64], f32, tag="ropet", bufs=2)
                    tb2 = p2.tile([128, 2 * HL, 64], f32, tag="ropet2", bufs=2)
                    nc.vector.tensor_tensor(ta2, qkn4[:, :, 0], cs[:, 1], OP.mult)
                    nc.vector.tensor_tensor(tb2, qkn4[:, :, 1], cs[:, 0], OP.mult)
                    nc.vector.tensor_tensor(qkr4[:, :, 1], ta2, tb2, OP.add)

                    for hh in range(HL):
                        nc.vector.tensor_copy(
                            tl_V[:, t, hh, 0:HD],
                            tl_qkvS[:, t, 2 * CHL + HD * hh:2 * CHL + HD * (hh + 1)])
                    nc.vector.memset(tl_V[:, t, :, HD:HD + 1], 1.0)

                    for hh in range(HL):
                        nc.sync.dma_start(
                            out=tl_QT[:, hh, 128 * t:128 * (t + 1)],
                            in_=qkr[:, 0, hh], transpose=True)
                        nc.sync.dma_start(
                            out=tl_KT[:, hh, 128 * t:128 * (t + 1)],
                            in_=qkr[:, 1, hh], transpose=True)

            # ---------------- P4: attention ----------------
            SQC = (512, 512, 256)
            with pool(name="pTp", bufs=2) as pTp, \
                 pool(name="psum4", bufs=2, space="PSUM") as psum4:
                for hh in range(HL):
                    tl_pT = pTp.tile([128, NT, S], bf16, tag="pT")
                    for t in range(NT):
                        off = 0
                        for qc in SQC:
                            ps = psum4.tile([128, 512], f32, tag="psc")
                            nc.tensor.matmul(
                                ps[:, :qc], tl_KT[:, hh, 128 * t:128 * (t + 1)],
                                tl_QT[:, hh, off:off + qc],
                                start=True, stop=True)
                            nc.scalar.activation(tl_pT[:, t, off:off + qc],
                                                 ps[:, :qc], AF.Exp,
                                                 bias=tl_mb[:, t:t + 1],
                                                 scale=INV_SQRT_HD)
                            off += qc
                    for m in range(NT):
                        po = psum4.tile([128, HD + 1], f32, tag="pav")
                        for t in range(NT):
                            nc.tensor.matmul(po,
                                             tl_pT[:, t, 128 * m:128 * (m + 1)],
                                             tl_V[:, t, hh], start=(t == 0),
                                             stop=(t == NT - 1))
                        rcp = small.tile([128, 1], f32, tag="rcp")
                        nc.vector.reciprocal(rcp, po[:, HD:HD + 1])
                        nc.vector.tensor_scalar_mul(
                            tl_attn[:, m, HD * hh:HD * (hh + 1)],
                            po[:, 0:HD], rcp)

                for m in range(NT):
                    for hh in range(HL):
                        nc.sync.dma_start(
                            out=tl_attnT[:, hh, 128 * m:128 * (m + 1)],
                            in_=tl_attn[:, m, HD * hh:HD * (hh + 1)],
                            transpose=True)

            # ---------------- P5: to_out partials -> RS ----------------
            with pool(name="p5", bufs=1) as p5, \
                 pool(name="psum5", bufs=4, space="PSUM") as psum5:
                tl_wo = p5.tile([128, HL, D], bf16)
                tl_wao = p5.tile([128, HL, D], bf16)
                nc.sync.dma_start(
                    out=tl_wo, in_=toout_wT[:].rearrange("(h p) x -> p h x", p=128))
                nc.sync.dma_start(
                    out=tl_wao, in_=taout_wT[:].rearrange("(h p) x -> p h x", p=128))
                for t in range(NT):
                    wo = tl_wao if t < NT_TXT else tl_wo
                    outb = p5.tile([128, D], bf16, tag="outb", bufs=2, name="outb")
                    for n in range(6):
                        pp = psum5.tile([128, 512], f32, tag="po")
                        for k in range(HL):
                            nc.tensor.matmul(
                                pp, tl_attnT[:, k, 128 * t:128 * (t + 1)],
                                wo[:, k, 512 * n:512 * (n + 1)],
                                start=(k == 0), stop=(k == HL - 1))
                        nc.scalar.copy(outb[:, 512 * n:512 * (n + 1)], pp)
                    nc.gpsimd.dma_start(out=tile_rows(attn_rs_in, t), in_=outb)
                collective("ReduceScatter", OP.add, attn_rs_in, attn_rs_out)

        # =================== P6 / P7 / P8 ===============================
        with pool(name="p68", bufs=1) as p68:
            tl_h2_t = p68.tile([32, D], f32)
            tl_h2_i = p68.tile([128, D], f32)

            with pool(name="pl6", bufs=2) as pl6:
                for (parts, rsl, stream, bvec, h2dst) in (
                        (32, slice(0, 32), 1, taout_b, tl_h2_t),
                        (128, slice(32, 160), 0, toout_b, tl_h2_i)):
                    hsh = pl6.tile([parts, D], f32, tag="big32b", bufs=2, name="hsh6")
                    nc.gpsimd.dma_start(out=hsh, in_=h_shard[rsl])
                    a = pl6.tile([parts, D], bf16, tag="bigbf", bufs=2, name="ars")
                    nc.gpsimd.dma_start(out=a, in_=attn_rs_out[rsl])
                    bb = vec_bcast(pl6, bvec, parts)
                    g1 = plane_bcast(pl6, stream, 2, parts)
                    t1 = pl6.tile([parts, D], f32, tag="big32", bufs=2, name="t1")
                    nc.vector.tensor_tensor(t1, a, bb, OP.add)
                    t2 = pl6.tile([parts, D], f32, tag="big32c", bufs=2, name="t2")
                    nc.vector.tensor_tensor(t2, t1, g1, OP.mult)
                    nc.vector.tensor_tensor(h2dst, hsh, t2, OP.add)
                    lnx = pl6.tile([parts, D], f32, tag="big32", bufs=2, name="lnx6")
                    layernorm(lnx, h2dst, parts)
                    sc = plane_bcast(pl6, stream, 4, parts)
                    sh = plane_bcast(pl6, stream, 3, parts)
                    x2b = pl6.tile([parts, D], bf16, tag="bigbf", bufs=2, name="x2b")
                    modulate(pl6, x2b, lnx, sc, sh, parts)
                    nc.gpsimd.dma_start(out=x2_ag_in[rsl], in_=x2b)
                collective("AllGather", OP.bypass, x2_ag_in, x2_full)

            # ---------------- P7: MLP ----------------
            with pool(name="p7", bufs=1) as p7, \
                 pool(name="psum7", bufs=8, space="PSUM") as psum7:
                tl_x2T = p7.tile([128, NKT, S], bf16)
                for k in range(NKT):
                    nc.sync.dma_start(out=tl_x2T[:, k],
                                      in_=x2_full[:, 128 * k:128 * (k + 1)],
                                      transpose=True)
                tl_x2Tt = p7.tile([128, NKT, 256], bf16)
                gather_txt(tl_x2T, tl_x2Tt)
                tl_b1i = p7.tile([128, FFL], bf16)
                nc.gpsimd.dma_start(out=tl_b1i,
                                    in_=bc_ap(b1i, 0, [[0, 128], [1, FFL]]))
                tl_b1t = p7.tile([128, FFL], bf16)
                nc.gpsimd.dma_start(out=tl_b1t,
                                    in_=bc_ap(b1t, 0, [[0, 128], [1, FFL]]))
                tl_gT = p7.tile([128, NF, S], bf16)

                for stream in (0, 1):
                    w1T, w2T, bt = ((w1iT, w2iT, tl_b1i) if stream == 0
                                    else (w1tT, w2tT, tl_b1t))
                    tiles = list(range(NT_TXT, NT)) if stream == 0 \
                        else list(range(NT_TXT))
                    for n in range(3):
                        pgs = [psum7.tile([128, 512], f32, tag="pm",
                                          name=f"pg_{stream}_{n}_{i}")
                               for i in range(len(tiles))]
                        for k in range(NKT):
                            wt = wstream.tile([128, 512], bf16, tag="w")
                            nc.sync.dma_start(
                                out=wt, in_=w1T[128 * k:128 * (k + 1),
                                               512 * n:512 * (n + 1)])
                            for i, t in enumerate(tiles):
                                nc.tensor.matmul(pgs[i],
                                                 xT_cols(tl_x2T, tl_x2Tt, k, t), wt,
                                                 start=(k == 0),
                                                 stop=(k == NKT - 1))
                        for i, t in enumerate(tiles):
                            pre = p7.tile([128, 512], f32, tag="pre", bufs=3)
                            nc.vector.tensor_tensor(
                                pre, pgs[i], bt[:, 512 * n:512 * (n + 1)],
                                OP.add)
                            g = p7.tile([128, 512], bf16, tag="g", bufs=3)
                            if sim:
                                # tanh-gelu via primitive ops (CoreSim path)
                                u = p7.tile([128, 512], f32, tag="gu", bufs=2,
                                            name="gu")
                                nc.scalar.activation(u, pre, AF.Square)
                                nc.vector.tensor_tensor(u, u, pre, OP.mult)
                                nc.vector.scalar_tensor_tensor(
                                    u, u, 0.044715, pre, OP.mult, OP.add)
                                nc.scalar.activation(u, u, AF.Tanh,
                                                     scale=0.7978845608028654)
                                xh = p7.tile([128, 512], f32, tag="gxh", bufs=2,
                                             name="gxh")
                                nc.scalar.mul(xh, pre, 0.5)
                                nc.vector.scalar_tensor_tensor(
                                    u, u, 1.0, xh, OP.add, OP.mult)
                                nc.vector.tensor_copy(g, u)
                            else:
                                nc.scalar.activation(g, pre, AF.Gelu_apprx_tanh)
                            for j in range(4):
                                nc.sync.dma_start(
                                    out=tl_gT[:, 4 * n + j,
                                              128 * t:128 * (t + 1)],
                                    in_=g[:, 128 * j:128 * (j + 1)],
                                    transpose=True)
                    for n in range(6):
                        pps = [psum7.tile([128, 512], f32, tag="pm",
                                          name=f"pp_{stream}_{n}_{i}")
                               for i in range(len(tiles))]
                        for k in range(NF):
                            wt = wstream.tile([128, 512], bf16, tag="w")
                            nc.sync.dma_start(
                                out=wt, in_=w2T[128 * k:128 * (k + 1),
                                               512 * n:512 * (n + 1)])
                            for i, t in enumerate(tiles):
                                nc.tensor.matmul(
                                    pps[i], tl_gT[:, k, 128 * t:128 * (t + 1)],
                                    wt, start=(k == 0), stop=(k == NF - 1))
                        for i, t in enumerate(tiles):
                            st = p7.tile([128, 512], bf16, tag="st", bufs=4)
                            nc.scalar.copy(st, pps[i])
                            nc.gpsimd.dma_start(
                                out=tile_rows(mlp_rs_in, t,
                                              slice(512 * n, 512 * (n + 1))),
                                in_=st)
                collective("ReduceScatter", OP.add, mlp_rs_in, mlp_rs_out)

            # ---------------- P8: final ----------------
            with pool(name="pl8", bufs=2) as pl8:
                for (parts, rsl, stream, bvec, h2src) in (
                        (32, slice(0, 32), 1, b2t, tl_h2_t),
                        (128, slice(32, 160), 0, b2i, tl_h2_i)):
                    m = pl8.tile([parts, D], bf16, tag="bigbf", bufs=2, name="mrs")
                    nc.gpsimd.dma_start(out=m, in_=mlp_rs_out[rsl])
                    bb = vec_bcast(pl8, bvec, parts)
                    g2 = plane_bcast(pl8, stream, 5, parts)
                    t1 = pl8.tile([parts, D], f32, tag="big32", bufs=2, name="t18")
                    nc.vector.tensor_tensor(t1, m, bb, OP.add)
                    t2 = pl8.tile([parts, D], f32, tag="big32c", bufs=2, name="t28")
                    nc.vector.tensor_tensor(t2, t1, g2, OP.mult)
                    fin = pl8.tile([parts, D], f32, tag="big32b", bufs=2, name="fin")
                    nc.vector.tensor_tensor(fin, h2src, t2, OP.add)
                    nc.gpsimd.dma_start(out=out_shard[rsl], in_=fin)

    nc.compile()
    return nc


# ------------------------------------------------------------- host wrappers
_CACHE = {}


def _prep_in_maps(inputs):
    inp = {k: np.asarray(v) for k, v in inputs.items()}
    hs = inp["hidden_states"][0].astype(np.float32)
    ehs = inp["encoder_hidden_states"][0].astype(np.float32)
    mask = inp["encoder_hidden_states_mask"][0].astype(np.float32)
    temb = inp["temb"][0].astype(np.float32)
    X = np.ascontiguousarray(np.concatenate([ehs, hs], 0))

    mask_bias = np.where(mask > 0, 0.0, np.float32(-30000.0)).astype(np.float32)
    mb_t = np.concatenate([mask_bias, np.zeros(S_IMG, np.float32)])

    def bfc(x):
        return np.ascontiguousarray(x.astype(BF16))

    def f32c(x):
        return np.ascontiguousarray(x.astype(np.float32))

    in_maps = []
    for c in range(NC):
        mrows = np.array([6 * (CHL * c + j) + p for p in range(6)
                          for j in range(CHL)])
        qkv_rows = []
        for base, perm in ((0, True), (D, True), (2 * D, False)):
            for h in range(HL * c, HL * c + HL):
                d = HD_PERM if perm else np.arange(HD)
                qkv_rows += list(base + h * HD + d)
        qkv_rows = np.array(qkv_rows)

        im = dict(
            h_shard=f32c(X[SEQ_PERM[160 * c:160 * c + 160]]),
            temb_in=f32c(temb),
            ropec=f32c(inp["rope_cos"]),
            ropes=f32c(inp["rope_sin"]),
            mask_b=f32c(mb_t),
            mod_wT=bfc(np.concatenate([inp["img_mod_w"][mrows],
                                       inp["txt_mod_w"][mrows]], 0).T),
            mod_b=f32c(np.concatenate([inp["img_mod_b"][mrows],
                                       inp["txt_mod_b"][mrows]])),
            qkv_wT=bfc(inp["qkv_w"][qkv_rows].T),
            qkv_b=f32c(inp["qkv_b"][qkv_rows]),
            aqkv_wT=bfc(inp["add_qkv_w"][qkv_rows].T),
            aqkv_b=f32c(inp["add_qkv_b"][qkv_rows]),
            nqw=f32c(inp["norm_q_w"][HD_PERM]),
            nkw=f32c(inp["norm_k_w"][HD_PERM]),
            naqw=f32c(inp["norm_added_q_w"][HD_PERM]),
            nakw=f32c(inp["norm_added_k_w"][HD_PERM]),
            toout_wT=bfc(inp["to_out_w"][:, CHL * c:CHL * (c + 1)].T),
            taout_wT=bfc(inp["to_add_out_w"][:, CHL * c:CHL * (c + 1)].T),
            toout_b=f32c(inp["to_out_b"]),
            taout_b=f32c(inp["to_add_out_b"]),
            w1iT=bfc(inp["img_mlp_w1"][FFL * c:FFL * (c + 1)].T),
            w1tT=bfc(inp["txt_mlp_w1"][FFL * c:FFL * (c + 1)].T),
            b1i=f32c(inp["img_mlp_b1"][FFL * c:FFL * (c + 1)]),
            b1t=f32c(inp["txt_mlp_b1"][FFL * c:FFL * (c + 1)]),
            w2iT=bfc(inp["img_mlp_w2"][:, FFL * c:FFL * (c + 1)].T),
            w2tT=bfc(inp["txt_mlp_w2"][:, FFL * c:FFL * (c + 1)].T),
            b2i=f32c(inp["img_mlp_b2"]),
            b2t=f32c(inp["txt_mlp_b2"]),
        )
        in_maps.append(im)
    return in_maps


def _run(in_maps, **kw):
    from concourse.bass_utils import run_bass_kernel_spmd

    if "nc" not in _CACHE:
        _CACHE["nc"] = _build_program()
    return run_bass_kernel_spmd(_CACHE["nc"], in_maps, list(range(NC)), **kw)


def kernel(**inputs):
    res = _run(_prep_in_maps(inputs))
    out = np.empty((S, D), np.float32)
    for c in range(NC):
        out[SEQ_PERM[160 * c:160 * c + 160]] = res.results[c]["out_shard"]
    return out[None, :S_TXT].copy(), out[None, S_TXT:].copy()


if __name__ == "__main__":
    _build_program()
    print("build ok")
